# revision 17
# baseline (speedup 1.0000x reference)
"""Trainium2 Bass kernel for a per-head dense MLP (CriticCVaR head).

Computes, per head t:
    h   = silu(states[t] @ W1[t] + b1[t])        # [B, S] @ [S, H]
    out = (h @ W2[t] + b2[t]).squeeze(-1)        # [B, H] @ [H, 1] -> [B]

Sharding: heads T=32 split across 8 NeuronCores (4 heads/core, full batch).

Device layout / schedule:
  - states are pre-transposed on the host to [S, B] and shipped as
    fp8e3 (e3m4): the PE accepts a mixed-dtype matmul (fp16 stationary
    W1 x fp8e3 moving X) at full rate, so X DMA traffic halves while
    the W1 operand keeps fp16 precision (measured end-to-end rel err
    ~1.4e-2 vs the 2e-2 budget).
  - the batch is processed in column blocks of BW; per block the PE
    stream is MM1(blk, t0..t3) then MM2(blk-1): the second matmul runs
    one block behind so its dependency on silu(z) is always satisfied
    and the PE never idles waiting on the activation engine.
  - the four heads' M=1 second matmuls are col-tiled (tile_position)
    onto partitions 0/32/64/96 of one PSUM tile so the bias-add + PSUM
    evacuation is one multi-lane DVE op.
  - X rides the sync HWDGE ring with one trigger per (blk, t, k) in
    consumption order; consts ride the scalar ring; output stores use
    the GPSIMD SWDGE path.
"""

from contextlib import ExitStack

import numpy as np

T, B, S, H = 32, 8192, 256, 128
NCORES = 8
TLOC = T // NCORES          # heads per core
KCH = S // 128              # contraction chunks (S on partitions)
MMN = 512                   # matmul free dim (one PSUM bank of fp32)
BW = 1024                   # batch columns per pipeline block


def build_nc(b_total: int = B, bw: int = BW, use_silu: bool = True):
    import concourse.mybir as mybir
    import concourse.tile as tile
    from concourse import bacc

    fp16 = mybir.dt.float16
    fp32 = mybir.dt.float32
    f83 = mybir.dt.float8e3
    nbb = b_total // bw

    nc = bacc.Bacc("TRN2", target_bir_lowering=False, debug=False)
    xT = nc.dram_tensor("xT", [TLOC, KCH, 128, b_total], f83, kind="ExternalInput")
    w1 = nc.dram_tensor("w1", [128, TLOC * KCH * H], fp16, kind="ExternalInput")
    b1 = nc.dram_tensor("b1", [H, TLOC], fp32, kind="ExternalInput")
    w2 = nc.dram_tensor("w2", [H, 32 * TLOC], fp16, kind="ExternalInput")
    b2 = nc.dram_tensor("b2", [128, 1], fp32, kind="ExternalInput")  # b2[t] at row 32t
    out = nc.dram_tensor("out", [TLOC, b_total], fp32, kind="ExternalOutput")

    silu = mybir.ActivationFunctionType.Silu

    with ExitStack() as ctx:
        tc = ctx.enter_context(tile.TileContext(nc))
        cpool = ctx.enter_context(tc.tile_pool(name="const", bufs=1))
        xpool = ctx.enter_context(tc.tile_pool(name="x", bufs=1))
        zpool = ctx.enter_context(tc.tile_pool(name="z", bufs=2))
        spool = ctx.enter_context(tc.tile_pool(name="s", bufs=2))
        opool = ctx.enter_context(tc.tile_pool(name="o", bufs=3))
        p1pool = ctx.enter_context(tc.tile_pool(name="p1", bufs=2, space="PSUM"))
        p2pool = ctx.enter_context(tc.tile_pool(name="p2", bufs=2, space="PSUM"))

        # Consts ride the scalar ring (issued before any silu queues up) so
        # the sync ring starts streaming X immediately. w1 is split so the
        # first matmul only waits on head 0's 64KB slice.
        w1sb = cpool.tile([128, TLOC * KCH * H], fp16)
        nc.scalar.dma_start(w1sb[:, 0 : KCH * H], w1.ap()[:, 0 : KCH * H])
        b1sb = cpool.tile([H, TLOC], fp32)
        nc.scalar.dma_start(b1sb[:, :], b1.ap()[:, :])
        w2sb = cpool.tile([H, 32 * TLOC], fp16)
        nc.scalar.dma_start(w2sb[:, :], w2.ap()[:, :])
        b2sb = cpool.tile([128, 1], fp32)
        nc.scalar.dma_start(b2sb[:, :], b2.ap()[:, :])
        nc.scalar.dma_start(w1sb[:, KCH * H :], w1.ap()[:, KCH * H :])

        # Warm-up ops: absorb the const-DMA waits and pre-load the Silu
        # activation table before the steady-state loop.
        warm_a = cpool.tile([H, TLOC], fp32)
        nc.scalar.activation(
            warm_a[:, :],
            b1sb[:, :],
            silu if use_silu else mybir.ActivationFunctionType.Sigmoid,
        )
        warm_v = cpool.tile([128, 1], fp32)
        nc.vector.tensor_scalar_add(warm_v[:, :], b1sb[:, 0:1], 0.0)

        # Whole-core X resident in SBUF: one persistent tile per (t, k),
        # filled by per-block column-chunk DMAs in consumption order so
        # early matmuls only wait on their own chunk.
        xtiles = {}
        for t in range(TLOC):
            for k in range(KCH):
                xtiles[t, k] = xpool.tile(
                    [128, b_total], f83, tag=f"x{t}{k}", name=f"xt{t}{k}"
                )
        # All X rides the sync HWDGE ring (the scalar queue must stay clear:
        # ACT has exec-queue depth 0, so a DMA trigger queued behind a silu
        # stalls until the silu finishes). Chunks are emitted in consumption
        # order -- fine at the start (pipeline ramp-up), coarse afterwards
        # to keep the ~650ns/trigger issue cost ahead of the PE.

        def x_chunks():
            # (t, k, lo, sz) in rough consumption order; block 0 head 0 is
            # k-interleaved at 512 to match the k-inner matmul order there
            for lo in (0, 512):
                for k in range(KCH):
                    yield 0, k, lo, 512
            for t in range(1, TLOC):
                for k in range(KCH):
                    yield t, k, 0, bw
            c0 = bw
            while c0 < b_total:
                sz = bw if c0 < 2 * bw else (2 * bw if c0 < 4 * bw else 4 * bw)
                for t in range(TLOC):
                    for k in range(KCH):
                        yield t, k, c0, sz
                c0 += sz

        for t, k, lo, sz in x_chunks():
            nc.sync.dma_start(
                xtiles[t, k][:, lo : lo + sz],
                xT.ap()[t, k, :, lo : lo + sz],
            )

        def mm1_block(bb, mm2=None):
            c0 = bb * bw
            for t in range(TLOC):
                # Inject the previous block's MM2 quadrant groups between
                # heads: after t1 the prior block's silu chain has finished,
                # and emitting each 4-matmul group contiguously (with only
                # MM1s around it) keeps the scheduler from splitting the
                # group, which would forfeit PE column-tile parallelism.
                if mm2 is not None and t in (2, 3):
                    mm2(t - 2)
                p1 = p1pool.tile([128, bw], fp32, tag="p1")
                # k-inner for the very first tile (chunks arrive k-interleaved
                # there), k-outer elsewhere (one weight serves both halves)
                if bb == 0 and t == 0:
                    order = [(k, hh) for hh in range(bw // MMN) for k in range(KCH)]
                else:
                    order = [(k, hh) for k in range(KCH) for hh in range(bw // MMN)]
                for k, hh in order:
                    hc = hh * MMN
                    nc.tensor.matmul(
                        p1[:, hc : hc + MMN],
                        w1sb[:, (t * KCH + k) * H : (t * KCH + k + 1) * H],
                        xtiles[t, k][:, c0 + hc : c0 + hc + MMN],
                        start=(k == 0),
                        stop=(k == KCH - 1),
                    )
                z = zpool.tile([128, bw], fp16, tag=f"z{t}")
                if use_silu:
                    nc.scalar.activation(
                        z[:, :], p1[:, :], silu, bias=b1sb[:, t : t + 1]
                    )
                else:
                    # CoreSim fallback: silu(y) = y * sigmoid(y)
                    sg = spool.tile([128, bw], fp16, tag="sg")
                    nc.scalar.activation(
                        sg[:, :],
                        p1[:, :],
                        mybir.ActivationFunctionType.Sigmoid,
                        bias=b1sb[:, t : t + 1],
                    )
                    yb = spool.tile([128, bw], fp32, tag="yb")
                    nc.vector.tensor_scalar_add(
                        yb[:, :], p1[:, :], b1sb[:, t : t + 1]
                    )
                    nc.vector.tensor_mul(z[:, :], yb[:, :], sg[:, :])
                zs[t] = z

        def mm2_group(bb, zprev, hh, esz=MMN):
            """One 512-col group: 4 quadrant matmuls + evacuation + store."""
            c0 = bb * bw
            hc = hh * MMN
            if hh == 0:
                p2s[bb] = p2pool.tile([128, bw], fp32, tag="p2", name="p2")
            p2 = p2s[bb]
            for t in range(TLOC):
                # M=32 with w2[t] replicated across columns: all rows of
                # the col-group get the head's result (same N-cycle cost
                # as M=1) so the PSUM tile is fully initialized.
                nc.tensor.matmul(
                    p2[32 * t : 32 * t + 32, hc : hc + MMN],
                    w2sb[:, 32 * t : 32 * t + 32],
                    zprev[t][:, hc : hc + MMN],
                    start=True,
                    stop=True,
                    tile_position=(0, 32 * t),
                )
            o = opool.tile([128, MMN], fp32, tag="o", name="o")
            for ec in range(0, MMN, esz):
                nc.vector.tensor_scalar_add(
                    o[:, ec : ec + esz],
                    p2[:, hc + ec : hc + ec + esz],
                    b2sb[:, 0:1],
                )
                nc.gpsimd.dma_start(
                    out.ap()[:, c0 + hc + ec : c0 + hc + ec + esz],
                    o[0:97:32, ec : ec + esz],
                )

        zs = {}
        p2s = {}
        zprev = None
        for bb in range(nbb):
            if zprev is not None:
                zp = zprev
                mm1_block(bb, mm2=lambda hh: mm2_group(bb - 1, zp, hh))
            else:
                mm1_block(bb)
            zprev = dict(zs)
        for hh in range(bw // MMN):
            mm2_group(nbb - 1, zprev, hh, esz=256)

    nc.compile()
    return nc


def make_in_maps(states_batch, W1, b1, W2, b2):
    import ml_dtypes

    states_batch = np.asarray(states_batch)
    W1, b1, W2, b2 = (np.asarray(a) for a in (W1, b1, W2, b2))
    b_total = states_batch.shape[1]
    in_maps = []
    for c in range(NCORES):
        sl = slice(c * TLOC, (c + 1) * TLOC)
        xT = (
            states_batch[sl]
            .transpose(0, 2, 1)
            .astype(ml_dtypes.float8_e3m4)
            .reshape(TLOC, KCH, 128, b_total)
        )
        w1h = (
            W1[sl]
            .reshape(TLOC, KCH, 128, H)
            .transpose(2, 0, 1, 3)
            .reshape(128, TLOC * KCH * H)
            .astype(np.float16)
        )
        b1h = np.ascontiguousarray(b1[sl].T).astype(np.float32)
        w2h = np.repeat(
            np.ascontiguousarray(W2[sl][:, :, 0].T).astype(np.float16), 32, axis=1
        )
        b2h = np.repeat(b2[sl, 0].astype(np.float32), 32).reshape(128, 1)
        in_maps.append({"xT": xT, "w1": w1h, "b1": b1h, "w2": w2h, "b2": b2h})
    return in_maps


def run(inputs: dict, trace: bool = False):
    from concourse import bass_utils

    nc = build_nc()
    in_maps = make_in_maps(**inputs)
    res = bass_utils.run_bass_kernel_spmd(
        nc, in_maps, core_ids=list(range(NCORES)), trace=trace
    )
    out = np.concatenate([r["out"] for r in res.results], axis=0)
    return out, res


def kernel(**inputs) -> np.ndarray:
    out, _ = run(inputs)
    return out
